# revision 22
# baseline (speedup 1.0000x reference)
"""Trainium2 Bass kernel for EnhancedStrategySuperposition (MoE soft routing).

Math (per token b):
    logits = x @ W_att.T + b_att + adaptive_bias          [B, E]
    w      = softmax(logits + gumbel(u))                  [B, E]
    y[e]   = x @ W_strat[e].T + b_strat[e]                [B, E, A]
    out    = sum_e w[:, e] * y[e]                         [B, A]

Strategy:
  - Data-parallel: batch B=8192 sharded across 8 cores (1024 tokens each);
    gating + strategy weights replicated.  All inputs partition-major fp16
    (x, W transposed to contract along the partition dim).
  - The PE runs one long gapless fp16 matmul stream at the full 2.4 GHz
    rate (216ns per [128x128]@[128x512]).  Two rules make that happen:
    (1) consecutive matmuls always accumulate into *different* PSUM banks
    (same-bank back-to-back accumulation costs ~46ns/matmul of RAW bubble),
    so strategy blocks process two token tiles interleaved; (2) a bf16
    warm-up stream occupies the PE from ~0.5us until the first x half
    lands, keeping the p-state ramp warm (idle gaps drop the clock to
    1.2 GHz for ~3us).
  - DMA order: wa (gating weights), x half 0, gumbel g, W group 0, x half
    1, then the remaining W groups (coarse chunks).  Gating for half 0
    runs while W group 0 is still landing; the strategy stream starts the
    moment group 0 arrives and never waits on DMA again.
  - Softmax keeps *unnormalized* exp weights u (logits+g <= ~28 so no
    max-subtract; exp row-sums via ACT accum_out); the 1/sum scale is
    folded into one final ScalarE copy per tile, saving DVE work in the
    drain loop.
  - Drains: ScalarE copies each finished PSUM bank to SBUF; DVE applies
    4 scalar_tensor_tensor FMAs per bank (per-partition scalar = u[:,e])
    into two alternating accumulators (breaks the RAW chain); trailing
    one block behind the matmul stream.  Output is staged partition-major
    and stored with 2 large DMAs; the host undoes the tile-major layout.
"""

import numpy as np

_B, _D, _E, _A = 8192, 1024, 32, 128
_NCORES = 8
_BL = _B // _NCORES  # tokens per core
_EPS = 1e-10

_KC = _D // 128  # contraction chunks
_JT = _BL // 128  # token tiles per core
_GG = _E // 4  # expert groups (4 experts x 128 cols = 512)
_NWARM = 22  # bf16 warm-up matmuls (fill PE until x half 0 + W group 0 land)

_cache = {}


def _build(with_bias=True):
    """Build + compile the per-core Bass program (cached)."""
    key = ("nc", with_bias)
    if key in _cache:
        return _cache[key]

    from contextlib import ExitStack

    from concourse import bacc, mybir, tile
    from concourse.bass import ts
    from concourse.masks import make_identity

    bf16 = mybir.dt.bfloat16
    f16 = mybir.dt.float16
    f32 = mybir.dt.float32

    nc = bacc.Bacc("TRN2", debug=False, num_devices=_NCORES)

    KC, JT, GG = _KC, _JT, _GG
    HTOK = KC * 512  # x elements per half (per partition)

    xt_d = nc.dram_tensor("xt16", [128, 2 * HTOK], f16, kind="ExternalInput").ap()
    wt_d = nc.dram_tensor(
        "wt16", [128, GG * KC * 512], f16, kind="ExternalInput"
    ).ap()
    wa_d = nc.dram_tensor("wa16", [128, KC * _E], f16, kind="ExternalInput").ap()
    # gumbel noise transposed: gT[e, token] (token = h*512 + t within core)
    g_d = nc.dram_tensor("g32", [_E, _BL], f32, kind="ExternalInput").ap()
    bs_d = (
        nc.dram_tensor("bs32", [_E, _A], f32, kind="ExternalInput").ap()
        if with_bias
        else None
    )
    # output partition-major: out[p, j*A + a] = out_token(j*128+p, a)
    out_d = nc.dram_tensor("out", [128, JT * _A], f32, kind="ExternalOutput").ap()

    with tile.TileContext(nc) as tc, ExitStack() as ctx:
        singles = ctx.enter_context(tc.tile_pool(name="singles", bufs=1))
        sb_small = ctx.enter_context(tc.tile_pool(name="small", bufs=3))

        # --- resident inputs, in DMA priority order ---
        wabig = singles.tile([128, KC * _E], f16, tag="wabig")
        nc.sync.dma_start(out=wabig, in_=wa_d[:, :])
        xbig = singles.tile([128, 2 * HTOK], f16, tag="xbig")
        nc.sync.dma_start(out=xbig[:, 0:HTOK], in_=xt_d[:, 0:HTOK])
        # W group 0 in two halves so the strategy stream can start after the
        # first 4 k-chunks land (the second half lands before k=4 is reached)
        wbig = singles.tile([128, GG * KC * 512], f16, tag="wbig")
        nc.sync.dma_start(out=wbig[:, 0 : 4 * 512], in_=wt_d[:, 0 : 4 * 512])
        nc.sync.dma_start(
            out=wbig[:, 4 * 512 : KC * 512], in_=wt_d[:, 4 * 512 : KC * 512]
        )
        gT_all = singles.tile([_E, _BL], f32, tag="g")
        nc.sync.dma_start(out=gT_all, in_=g_d[:, :])
        nc.sync.dma_start(
            out=xbig[:, HTOK : 2 * HTOK], in_=xt_d[:, HTOK : 2 * HTOK]
        )
        nc.sync.dma_start(out=wbig[:, ts(1, KC * 512)], in_=wt_d[:, ts(1, KC * 512)])
        if with_bias:
            bs_sb = singles.tile([_E, _A], f32, tag="bs")
            nc.sync.dma_start(out=bs_sb, in_=bs_d[:, :])
        nc.sync.dma_start(
            out=wbig[:, 2 * KC * 512 : 8 * KC * 512],
            in_=wt_d[:, 2 * KC * 512 : 8 * KC * 512],
        )

        ident = singles.tile([128, 128], f32, tag="ident")
        make_identity(nc, ident)
        ident16 = singles.tile([128, 128], f16, tag="ident16")
        make_identity(nc, ident16)

        # --- PE warm-up: bf16 matmuls on a memset tile, alternating two
        # PSUM banks, from ~0.5us until the x half-0 DMA lands.  Keeps the
        # p-state ramp warm so gating + strategy run at 2.4 GHz from their
        # first instruction.
        warm_in = singles.tile([128, 512], bf16, tag="warmin")
        nc.vector.memset(warm_in, 0.25)
        warm_sink = singles.tile([1, 2], f32, tag="warmsink")
        with tc.tile_pool(name="pswarm", bufs=1, space="PSUM") as ps_warm:
            pwa = ps_warm.tile([128, 512], f32, tag="warma")
            pwb = ps_warm.tile([128, 512], f32, tag="warmb")
            for _ in range(_NWARM // 2):
                nc.tensor.matmul(pwa, warm_in[:, 0:128], warm_in, start=True, stop=True)
                nc.tensor.matmul(pwb, warm_in[:, 0:128], warm_in, start=True, stop=True)
            nc.vector.tensor_copy(warm_sink[0:1, 0:1], pwa[0:1, 0:1])
            nc.vector.tensor_copy(warm_sink[0:1, 1:2], pwb[0:1, 0:1])

        def x_lhsT(k, j):  # [128, 128] fp16: d-chunk k, token tile j
            h, t0 = j // 4, (j % 4) * 128
            return xbig[:, h * HTOK + k * 512 + t0 : h * HTOK + k * 512 + t0 + 128]

        usb = [
            singles.tile([128, _E], f32, tag=f"u{j}", name=f"u{j}") for j in range(JT)
        ]
        rinv = [
            singles.tile([128, 1], f32, tag=f"rinv{j}", name=f"rinv{j}")
            for j in range(JT)
        ]
        acca = [
            singles.tile([128, _A], f32, tag=f"acca{j}", name=f"acca{j}")
            for j in range(JT)
        ]
        accb = [
            singles.tile([128, _A], f32, tag=f"accb{j}", name=f"accb{j}")
            for j in range(JT)
        ]
        outsb = singles.tile([128, JT * _A], f32, tag="outsb")

        from contextlib import nullcontext

        with (
            tc.tile_pool(name="pgate", bufs=1, space="PSUM") as ps_gate,
            tc.tile_pool(name="pplg", bufs=1, space="PSUM") as ps_plg,
            (
                tc.tile_pool(name="pswt", bufs=1, space="PSUM")
                if with_bias
                else nullcontext()
            ) as ps_wt,
            tc.tile_pool(
                name="psbig", bufs=(2 if with_bias else 3), space="PSUM"
            ) as ps_big,
            tc.tile_pool(name="ybuf", bufs=4) as ybuf,
        ):

            def emit_gating(h):
                # logits^T for tokens [h*512, (h+1)*512): one DVE add fuses
                # the PSUM drain with the gumbel-noise add (gT is host-side
                # transposed), then per-tile PE transpose + exp-from-PSUM.
                plgT = ps_gate.tile([_E, 512], f32, tag="lgT", name="lgT")
                for k in range(KC):
                    nc.tensor.matmul(
                        plgT,
                        wabig[:, ts(k, _E)],
                        xbig[:, h * HTOK + k * 512 : h * HTOK + (k + 1) * 512],
                        start=(k == 0),
                        stop=(k == KC - 1),
                    )
                # f16 logits: 1 cycle/row PE transpose (f32 would be 2)
                lgT_sb = sb_small.tile([_E, 512], f16, tag="lgT_sb", name="lgT_sb")
                nc.vector.tensor_add(
                    lgT_sb, plgT, gT_all[:, h * 512 : (h + 1) * 512]
                )
                for t in range(4):
                    j = h * 4 + t
                    plg = ps_plg.tile([128, _E], f16, tag="lg", name="lg")
                    nc.tensor.transpose(
                        plg, lgT_sb[:, ts(t, 128)], ident16[:_E, :_E]
                    )
                    s = sb_small.tile([128, 1], f32, tag="s", name="s")
                    nc.scalar.activation(
                        usb[j],
                        plg,
                        mybir.ActivationFunctionType.Exp,
                        bias=0.0,
                        scale=1.0,
                        accum_out=s,
                    )
                    nc.vector.reciprocal(rinv[j], s)

            def emit_drain(gi, jp, ps_pair, direct=False):
                for which, j in enumerate((2 * jp, 2 * jp + 1)):
                    ps = ps_pair[which]
                    if gi == 0 and with_bias:
                        # seed acc with sum_e u[:,e] b_strat[e,:]
                        pwt = ps_wt.tile([_E, 128], f32, tag="pwt", name="pwt")
                        nc.tensor.transpose(pwt, usb[j], ident)
                        wt_sb = sb_small.tile(
                            [_E, 128], f32, tag="wt_sb", name="wt_sb"
                        )
                        nc.vector.tensor_copy(wt_sb, pwt)
                        pa0 = ps_wt.tile([128, _A], f32, tag="pa0", name="pa0")
                        nc.tensor.matmul(pa0, wt_sb, bs_sb, start=True, stop=True)
                        nc.vector.tensor_copy(acca[j], pa0)
                    if direct and which == 1:
                        ysb = ps  # very last bank: read PSUM directly
                    else:
                        ysb = ybuf.tile([128, 512], f32, tag="y", name="y")
                        nc.scalar.copy(ysb, ps)
                    for i in range(4):
                        e = gi * 4 + i
                        ucol = usb[j][:, e : e + 1]
                        if e == 1 or (e == 0 and not with_bias):
                            nc.vector.tensor_scalar_mul(
                                accb[j] if e == 1 else acca[j],
                                ysb[:, ts(i, 128)],
                                ucol,
                            )
                        else:
                            dst = acca[j] if e % 2 == 0 else accb[j]
                            nc.vector.scalar_tensor_tensor(
                                out=dst,
                                in0=ysb[:, ts(i, 128)],
                                scalar=ucol,
                                in1=dst,
                                op0=mybir.AluOpType.mult,
                                op1=mybir.AluOpType.add,
                            )
                    if gi == GG - 1:
                        nc.vector.tensor_add(acca[j], acca[j], accb[j])
                        # normalize by 1/sum(exp) in the staging copy
                        nc.scalar.activation(
                            outsb[:, ts(j, _A)],
                            acca[j],
                            mybir.ActivationFunctionType.Copy,
                            bias=0.0,
                            scale=rinv[j],
                        )
                if gi == GG - 1 and jp == 1:
                    nc.sync.dma_start(
                        out=out_d[:, 0 : 4 * _A], in_=outsb[:, 0 : 4 * _A]
                    )
                if gi == GG - 1 and jp == JT // 2 - 1:
                    nc.sync.dma_start(
                        out=out_d[:, 4 * _A : JT * _A],
                        in_=outsb[:, 4 * _A : JT * _A],
                    )

            # Strategy stream starts the moment x half 0 + W group 0 land;
            # gating is interleaved into the PE queue behind early blocks
            # (its results are only needed by the trailing drains).
            pending = None
            for gi in range(GG):
                for jp in range(JT // 2):
                    j0, j1 = 2 * jp, 2 * jp + 1
                    ps_a = ps_big.tile([128, 512], f32, tag="bankA", name="bankA")
                    ps_b = ps_big.tile([128, 512], f32, tag="bankB", name="bankB")
                    last = gi == GG - 1 and jp == JT // 2 - 1
                    # final block: finish bankA two k-steps early so its
                    # drain overlaps the stream tail
                    korder = (
                        [(k, w) for k in range(KC - 2) for w in (0, 1)]
                        + [(KC - 2, 0), (KC - 1, 0), (KC - 2, 1), (KC - 1, 1)]
                        if last
                        else [(k, w) for k in range(KC) for w in (0, 1)]
                    )
                    for k, w in korder:
                        wk = wbig[:, gi * KC * 512 + k * 512 : gi * KC * 512 + (k + 1) * 512]
                        nc.tensor.matmul(
                            ps_a if w == 0 else ps_b,
                            x_lhsT(k, j0 if w == 0 else j1),
                            wk,
                            start=(k == 0),
                            stop=(k == KC - 1),
                        )
                    if gi == 0 and jp == 0:
                        emit_gating(0)
                    if gi == 0 and jp == 2:
                        emit_gating(1)
                    if pending is not None:
                        emit_drain(*pending)
                    pending = (gi, jp, (ps_a, ps_b))
            emit_drain(*pending, direct=True)

    nc.compile()
    _cache[key] = nc
    return nc


def _prep_in_maps(
    x, W_att, b_att, adaptive_bias, W_strat, b_strat, gumbel_u, with_bias=True
):
    x = np.asarray(x, dtype=np.float32)
    W_att = np.asarray(W_att, dtype=np.float32)
    b_att = np.asarray(b_att, dtype=np.float32)
    adaptive_bias = np.asarray(adaptive_bias, dtype=np.float32)
    W_strat = np.asarray(W_strat, dtype=np.float32)
    b_strat = np.asarray(b_strat, dtype=np.float32)
    gumbel_u = np.asarray(gumbel_u, dtype=np.float32)

    KC, JT, GG = _KC, _JT, _GG

    # x: per core, half-major: X[p, h*(KC*512) + k*512 + t] = x[c*BL+h*512+t, k*128+p]
    x16 = x.astype(np.float16)

    # W_strat: WT[d, e*A+a]; grouped [p, gi, k, c] with c in [0,512)
    WT = W_strat.transpose(2, 0, 1).reshape(_D, _E * _A).astype(np.float16)
    Wb = (
        WT.reshape(KC, 128, GG, 512)
        .transpose(1, 2, 0, 3)
        .reshape(128, GG * KC * 512)
    )
    Wb = np.ascontiguousarray(Wb)

    # W_att: Wa[p, k*E + e] = W_att[e, k*128+p]
    Wa = np.ascontiguousarray(
        W_att.T.astype(np.float16).reshape(KC, 128, _E).transpose(1, 0, 2)
    ).reshape(128, KC * _E)

    bias_row = (b_att + adaptive_bias).astype(np.float32)
    g = -np.log(-np.log(gumbel_u + np.float32(_EPS)) + np.float32(_EPS))
    g = (g + bias_row[None, :]).astype(np.float32)  # [B, E]

    bs32 = np.ascontiguousarray(b_strat, dtype=np.float32)

    in_maps = []
    for c in range(_NCORES):
        sl = slice(c * _BL, (c + 1) * _BL)
        xc = np.ascontiguousarray(
            x16[sl].reshape(2, 512, KC, 128).transpose(3, 0, 2, 1)
        ).reshape(128, 2 * KC * 512)
        gc = np.ascontiguousarray(g[sl].T)  # [E, BL] transposed
        m = {
            "xt16": xc,
            "wt16": Wb,
            "wa16": Wa,
            "g32": gc,
        }
        if with_bias:
            m["bs32"] = bs32
        in_maps.append(m)
    return in_maps


def kernel(x, W_att, b_att, adaptive_bias, W_strat, b_strat, gumbel_u):
    assert x.shape == (_B, _D) and W_strat.shape == (_E, _A, _D)
    with_bias = bool(np.any(np.asarray(b_strat)))
    nc = _build(with_bias=with_bias)
    in_maps = _prep_in_maps(
        x, W_att, b_att, adaptive_bias, W_strat, b_strat, gumbel_u,
        with_bias=with_bias,
    )
    from concourse.bass_utils import run_bass_kernel_spmd

    res = None
    for attempt in range(3):
        try:
            res = run_bass_kernel_spmd(nc, in_maps, list(range(_NCORES))).results
            break
        except Exception:
            # Transient device errors clear after a reset; drop the backend
            # and rebuild it.
            if attempt == 2:
                raise
            import time

            import jax

            time.sleep(3.0 * (attempt + 1))
            try:
                jax.clear_backends()
            except Exception:
                pass
    # undo partition-major output layout: res[p, j*A+a] -> out[j*128+p, a]
    out = np.concatenate(
        [
            res[c]["out"].reshape(128, _JT, _A).transpose(1, 0, 2).reshape(_BL, _A)
            for c in range(_NCORES)
        ],
        axis=0,
    )
    return np.ascontiguousarray(out.astype(np.float32))


# revision 28
# speedup vs baseline: 1.0014x; 1.0014x over previous
"""Trainium2 Bass kernel for EnhancedStrategySuperposition (MoE soft routing).

Math (per token b):
    logits = x @ W_att.T + b_att + adaptive_bias          [B, E]
    w      = softmax(logits + gumbel(u))                  [B, E]
    y[e]   = x @ W_strat[e].T + b_strat[e]                [B, E, A]
    out    = sum_e w[:, e] * y[e]                         [B, A]

Strategy:
  - Data-parallel: batch B=8192 sharded across 8 cores (1024 tokens each);
    gating + strategy weights replicated.  All inputs partition-major fp16
    (x, W transposed to contract along the partition dim).
  - The PE runs one long gapless fp16 matmul stream at the full 2.4 GHz
    rate (216ns per [128x128]@[128x512]).  Two rules make that happen:
    (1) consecutive matmuls always accumulate into *different* PSUM banks
    (same-bank back-to-back accumulation costs ~46ns/matmul of RAW bubble),
    so strategy blocks process two token tiles interleaved; (2) a bf16
    warm-up stream occupies the PE from ~0.5us until the first x half
    lands, keeping the p-state ramp warm (idle gaps drop the clock to
    1.2 GHz for ~3us).
  - DMA order: wa (gating weights), x half 0, gumbel g, W group 0, x half
    1, then the remaining W groups (coarse chunks).  Gating for half 0
    runs while W group 0 is still landing; the strategy stream starts the
    moment group 0 arrives and never waits on DMA again.
  - Softmax keeps *unnormalized* exp weights u (logits+g <= ~28 so no
    max-subtract; exp row-sums via ACT accum_out); the 1/sum scale is
    folded into one final ScalarE copy per tile, saving DVE work in the
    drain loop.
  - Drains: ScalarE copies each finished PSUM bank to SBUF; DVE applies
    4 scalar_tensor_tensor FMAs per bank (per-partition scalar = u[:,e])
    into two alternating accumulators (breaks the RAW chain); trailing
    one block behind the matmul stream.  Output is staged partition-major
    and stored with 2 large DMAs; the host undoes the tile-major layout.
"""

import numpy as np

_B, _D, _E, _A = 8192, 1024, 32, 128
_NCORES = 8
_BL = _B // _NCORES  # tokens per core
_EPS = 1e-10

_KC = _D // 128  # contraction chunks
_JT = _BL // 128  # token tiles per core
_GG = _E // 4  # expert groups (4 experts x 128 cols = 512)
_NWARM = 7  # bf16 warm-up matmuls (fill PE until x half 0 starts landing)

_cache = {}


def _build(with_bias=True):
    """Build + compile the per-core Bass program (cached)."""
    key = ("nc", with_bias)
    if key in _cache:
        return _cache[key]

    from contextlib import ExitStack

    from concourse import bacc, mybir, tile
    from concourse.bass import ts
    from concourse.masks import make_identity

    bf16 = mybir.dt.bfloat16
    f16 = mybir.dt.float16
    f32 = mybir.dt.float32

    nc = bacc.Bacc("TRN2", debug=False, num_devices=_NCORES)

    KC, JT, GG = _KC, _JT, _GG
    HTOK = KC * 512  # x elements per half (per partition)

    xt_d = nc.dram_tensor("xt16", [128, 2 * HTOK], f16, kind="ExternalInput").ap()
    wt_d = nc.dram_tensor(
        "wt16", [128, GG * KC * 512], f16, kind="ExternalInput"
    ).ap()
    wa_d = nc.dram_tensor("wa16", [128, KC * _E], f16, kind="ExternalInput").ap()
    # gumbel noise transposed: gT[e, token] (token = h*512 + t within core)
    g_d = nc.dram_tensor("g32", [_E, _BL], f32, kind="ExternalInput").ap()
    bs_d = (
        nc.dram_tensor("bs32", [_E, _A], f32, kind="ExternalInput").ap()
        if with_bias
        else None
    )
    # output partition-major: out[p, j*A + a] = out_token(j*128+p, a)
    out_d = nc.dram_tensor("out", [128, JT * _A], f32, kind="ExternalOutput").ap()

    with tile.TileContext(nc) as tc, ExitStack() as ctx:
        singles = ctx.enter_context(tc.tile_pool(name="singles", bufs=1))
        sb_small = ctx.enter_context(tc.tile_pool(name="small", bufs=3))

        # --- resident inputs, in DMA priority order ---
        # x half 0 in two k-chunks so gating can start on k0-3 early; g before
        # W group 0 (the gating DVE-add needs it pre-stream); W group 0 in two
        # halves so the strategy stream starts after its first 4 k-chunks land
        wabig = singles.tile([128, KC * _E], f16, tag="wabig")
        nc.sync.dma_start(out=wabig, in_=wa_d[:, :])
        xbig = singles.tile([128, 2 * HTOK], f16, tag="xbig")
        nc.sync.dma_start(out=xbig[:, 0 : HTOK // 2], in_=xt_d[:, 0 : HTOK // 2])
        nc.sync.dma_start(out=xbig[:, HTOK // 2 : HTOK], in_=xt_d[:, HTOK // 2 : HTOK])
        gT_all = singles.tile([_E, _BL], f32, tag="g")
        nc.sync.dma_start(out=gT_all, in_=g_d[:, :])
        wbig = singles.tile([128, GG * KC * 512], f16, tag="wbig")
        nc.sync.dma_start(out=wbig[:, 0 : 4 * 512], in_=wt_d[:, 0 : 4 * 512])
        nc.sync.dma_start(
            out=wbig[:, 4 * 512 : KC * 512], in_=wt_d[:, 4 * 512 : KC * 512]
        )
        nc.sync.dma_start(
            out=xbig[:, HTOK : 2 * HTOK], in_=xt_d[:, HTOK : 2 * HTOK]
        )
        nc.sync.dma_start(out=wbig[:, ts(1, KC * 512)], in_=wt_d[:, ts(1, KC * 512)])
        if with_bias:
            bs_sb = singles.tile([_E, _A], f32, tag="bs")
            nc.sync.dma_start(out=bs_sb, in_=bs_d[:, :])
        nc.sync.dma_start(
            out=wbig[:, 2 * KC * 512 : 8 * KC * 512],
            in_=wt_d[:, 2 * KC * 512 : 8 * KC * 512],
        )

        ident = singles.tile([128, 128], f32, tag="ident")
        make_identity(nc, ident)
        ident16 = singles.tile([128, 128], f16, tag="ident16")
        make_identity(nc, ident16)

        # --- PE warm-up: bf16 matmuls on a memset tile, alternating two
        # PSUM banks, from ~0.5us until the x half-0 DMA lands.  Keeps the
        # p-state ramp warm so gating + strategy run at 2.4 GHz from their
        # first instruction.
        warm_in = singles.tile([128, 512], bf16, tag="warmin")
        nc.vector.memset(warm_in, 0.25)
        warm_sink = singles.tile([1, 2], f32, tag="warmsink")
        with tc.tile_pool(name="pswarm", bufs=1, space="PSUM") as ps_warm:
            pwa = ps_warm.tile([128, 512], f32, tag="warma")
            pwb = ps_warm.tile([128, 512], f32, tag="warmb")
            for _ in range(_NWARM // 2):
                nc.tensor.matmul(pwa, warm_in[:, 0:128], warm_in, start=True, stop=True)
                nc.tensor.matmul(pwb, warm_in[:, 0:128], warm_in, start=True, stop=True)
            nc.vector.tensor_copy(warm_sink[0:1, 0:1], pwa[0:1, 0:1])
            nc.vector.tensor_copy(warm_sink[0:1, 1:2], pwb[0:1, 0:1])

        def x_lhsT(k, j):  # [128, 128] fp16: d-chunk k, token tile j
            h, t0 = j // 4, (j % 4) * 128
            return xbig[:, h * HTOK + k * 512 + t0 : h * HTOK + k * 512 + t0 + 128]

        usb = [
            singles.tile([128, _E], f32, tag=f"u{j}", name=f"u{j}") for j in range(JT)
        ]
        rinv = [
            singles.tile([128, 1], f32, tag=f"rinv{j}", name=f"rinv{j}")
            for j in range(JT)
        ]
        acca = [
            singles.tile([128, _A], f32, tag=f"acca{j}", name=f"acca{j}")
            for j in range(JT)
        ]
        accb = [
            singles.tile([128, _A], f32, tag=f"accb{j}", name=f"accb{j}")
            for j in range(JT)
        ]
        outsb = singles.tile([128, JT * _A], f32, tag="outsb")

        from contextlib import nullcontext

        with (
            tc.tile_pool(name="pgate", bufs=1, space="PSUM") as ps_gate,
            tc.tile_pool(name="pplg", bufs=1, space="PSUM") as ps_plg,
            (
                tc.tile_pool(name="pswt", bufs=1, space="PSUM")
                if with_bias
                else nullcontext()
            ) as ps_wt,
            tc.tile_pool(
                name="psbig", bufs=(2 if with_bias else 3), space="PSUM"
            ) as ps_big,
            tc.tile_pool(name="ybuf", bufs=4) as ybuf,
        ):

            def emit_gating(h):
                # logits^T for tokens [h*512, (h+1)*512): one DVE add fuses
                # the PSUM drain with the gumbel-noise add (gT is host-side
                # transposed), then per-tile PE transpose + exp-from-PSUM.
                plgT = ps_gate.tile([_E, 512], f32, tag="lgT", name="lgT")
                for k in range(KC):
                    nc.tensor.matmul(
                        plgT,
                        wabig[:, ts(k, _E)],
                        xbig[:, h * HTOK + k * 512 : h * HTOK + (k + 1) * 512],
                        start=(k == 0),
                        stop=(k == KC - 1),
                    )
                # f16 logits: 1 cycle/row PE transpose (f32 would be 2)
                lgT_sb = sb_small.tile([_E, 512], f16, tag="lgT_sb", name="lgT_sb")
                nc.vector.tensor_add(
                    lgT_sb, plgT, gT_all[:, h * 512 : (h + 1) * 512]
                )
                for t in range(4):
                    j = h * 4 + t
                    plg = ps_plg.tile([128, _E], f16, tag="lg", name="lg")
                    nc.tensor.transpose(
                        plg, lgT_sb[:, ts(t, 128)], ident16[:_E, :_E]
                    )
                    s = sb_small.tile([128, 1], f32, tag="s", name="s")
                    nc.scalar.activation(
                        usb[j],
                        plg,
                        mybir.ActivationFunctionType.Exp,
                        bias=0.0,
                        scale=1.0,
                        accum_out=s,
                    )
                    nc.vector.reciprocal(rinv[j], s)

            def emit_drain(gi, jp, ps_pair, direct=False):
                for which, j in enumerate((2 * jp, 2 * jp + 1)):
                    ps = ps_pair[which]
                    if gi == 0 and with_bias:
                        # seed acc with sum_e u[:,e] b_strat[e,:]
                        pwt = ps_wt.tile([_E, 128], f32, tag="pwt", name="pwt")
                        nc.tensor.transpose(pwt, usb[j], ident)
                        wt_sb = sb_small.tile(
                            [_E, 128], f32, tag="wt_sb", name="wt_sb"
                        )
                        nc.vector.tensor_copy(wt_sb, pwt)
                        pa0 = ps_wt.tile([128, _A], f32, tag="pa0", name="pa0")
                        nc.tensor.matmul(pa0, wt_sb, bs_sb, start=True, stop=True)
                        nc.vector.tensor_copy(acca[j], pa0)
                    psum_direct = direct and which == 1
                    if psum_direct:
                        ysb = ps  # very last bank: read PSUM directly (DVE)
                    else:
                        ysb = ybuf.tile([128, 512], f32, tag="y", name="y")
                        nc.scalar.copy(ysb, ps)
                    for i in range(4):
                        e = gi * 4 + i
                        ucol = usb[j][:, e : e + 1]
                        eng = nc.vector
                        if e == 1 or (e == 0 and not with_bias):
                            eng.tensor_scalar_mul(
                                accb[j] if e == 1 else acca[j],
                                ysb[:, ts(i, 128)],
                                ucol,
                            )
                        else:
                            dst = acca[j] if e % 2 == 0 else accb[j]
                            eng.scalar_tensor_tensor(
                                out=dst,
                                in0=ysb[:, ts(i, 128)],
                                scalar=ucol,
                                in1=dst,
                                op0=mybir.AluOpType.mult,
                                op1=mybir.AluOpType.add,
                            )
                    if gi == GG - 1:
                        nc.vector.tensor_add(acca[j], acca[j], accb[j])
                        # normalize by 1/sum(exp) in the staging copy
                        nc.scalar.activation(
                            outsb[:, ts(j, _A)],
                            acca[j],
                            mybir.ActivationFunctionType.Copy,
                            bias=0.0,
                            scale=rinv[j],
                        )
                if gi == GG - 1 and jp == 1:
                    nc.sync.dma_start(
                        out=out_d[:, 0 : 4 * _A], in_=outsb[:, 0 : 4 * _A]
                    )
                if gi == GG - 1 and jp == JT // 2 - 1:
                    nc.sync.dma_start(
                        out=out_d[:, 4 * _A : JT * _A],
                        in_=outsb[:, 4 * _A : JT * _A],
                    )

            # Gating half 0 runs pre-stream (during the W group 0 DMA);
            # gating half 1 is interleaved into the PE queue behind an early
            # block (its results are only needed by the trailing drains).
            emit_gating(0)

            pending = None
            for gi in range(GG):
                for jp in range(JT // 2):
                    j0, j1 = 2 * jp, 2 * jp + 1
                    ps_a = ps_big.tile([128, 512], f32, tag="bankA", name="bankA")
                    ps_b = ps_big.tile([128, 512], f32, tag="bankB", name="bankB")
                    last = gi == GG - 1 and jp == JT // 2 - 1
                    # final block: finish bankA two k-steps early so its
                    # drain overlaps the stream tail
                    korder = (
                        [(k, w) for k in range(KC - 2) for w in (0, 1)]
                        + [(KC - 2, 0), (KC - 1, 0), (KC - 2, 1), (KC - 1, 1)]
                        if last
                        else [(k, w) for k in range(KC) for w in (0, 1)]
                    )
                    for k, w in korder:
                        wk = wbig[:, gi * KC * 512 + k * 512 : gi * KC * 512 + (k + 1) * 512]
                        nc.tensor.matmul(
                            ps_a if w == 0 else ps_b,
                            x_lhsT(k, j0 if w == 0 else j1),
                            wk,
                            start=(k == 0),
                            stop=(k == KC - 1),
                        )
                    if gi == 0 and jp == 2:
                        emit_gating(1)
                    if pending is not None:
                        emit_drain(*pending)
                    pending = (gi, jp, (ps_a, ps_b))
            emit_drain(*pending, direct=True)

    nc.compile()
    _cache[key] = nc
    return nc


def _prep_in_maps(
    x, W_att, b_att, adaptive_bias, W_strat, b_strat, gumbel_u, with_bias=True
):
    x = np.asarray(x, dtype=np.float32)
    W_att = np.asarray(W_att, dtype=np.float32)
    b_att = np.asarray(b_att, dtype=np.float32)
    adaptive_bias = np.asarray(adaptive_bias, dtype=np.float32)
    W_strat = np.asarray(W_strat, dtype=np.float32)
    b_strat = np.asarray(b_strat, dtype=np.float32)
    gumbel_u = np.asarray(gumbel_u, dtype=np.float32)

    KC, JT, GG = _KC, _JT, _GG

    # x: per core, half-major: X[p, h*(KC*512) + k*512 + t] = x[c*BL+h*512+t, k*128+p]
    x16 = x.astype(np.float16)

    # W_strat: WT[d, e*A+a]; grouped [p, gi, k, c] with c in [0,512)
    WT = W_strat.transpose(2, 0, 1).reshape(_D, _E * _A).astype(np.float16)
    Wb = (
        WT.reshape(KC, 128, GG, 512)
        .transpose(1, 2, 0, 3)
        .reshape(128, GG * KC * 512)
    )
    Wb = np.ascontiguousarray(Wb)

    # W_att: Wa[p, k*E + e] = W_att[e, k*128+p]
    Wa = np.ascontiguousarray(
        W_att.T.astype(np.float16).reshape(KC, 128, _E).transpose(1, 0, 2)
    ).reshape(128, KC * _E)

    bias_row = (b_att + adaptive_bias).astype(np.float32)
    g = -np.log(-np.log(gumbel_u + np.float32(_EPS)) + np.float32(_EPS))
    g = (g + bias_row[None, :]).astype(np.float32)  # [B, E]

    bs32 = np.ascontiguousarray(b_strat, dtype=np.float32)

    in_maps = []
    for c in range(_NCORES):
        sl = slice(c * _BL, (c + 1) * _BL)
        xc = np.ascontiguousarray(
            x16[sl].reshape(2, 512, KC, 128).transpose(3, 0, 2, 1)
        ).reshape(128, 2 * KC * 512)
        gc = np.ascontiguousarray(g[sl].T)  # [E, BL] transposed
        m = {
            "xt16": xc,
            "wt16": Wb,
            "wa16": Wa,
            "g32": gc,
        }
        if with_bias:
            m["bs32"] = bs32
        in_maps.append(m)
    return in_maps


def kernel(x, W_att, b_att, adaptive_bias, W_strat, b_strat, gumbel_u):
    assert x.shape == (_B, _D) and W_strat.shape == (_E, _A, _D)
    with_bias = bool(np.any(np.asarray(b_strat)))
    nc = _build(with_bias=with_bias)
    in_maps = _prep_in_maps(
        x, W_att, b_att, adaptive_bias, W_strat, b_strat, gumbel_u,
        with_bias=with_bias,
    )
    from concourse.bass_utils import run_bass_kernel_spmd

    res = None
    for attempt in range(3):
        try:
            res = run_bass_kernel_spmd(nc, in_maps, list(range(_NCORES))).results
            break
        except Exception:
            # Transient device errors clear after a reset; drop the backend
            # and rebuild it.
            if attempt == 2:
                raise
            import time

            import jax

            time.sleep(3.0 * (attempt + 1))
            try:
                jax.clear_backends()
            except Exception:
                pass
    # undo partition-major output layout: res[p, j*A+a] -> out[j*128+p, a]
    out = np.concatenate(
        [
            res[c]["out"].reshape(128, _JT, _A).transpose(1, 0, 2).reshape(_BL, _A)
            for c in range(_NCORES)
        ],
        axis=0,
    )
    return np.ascontiguousarray(out.astype(np.float32))
